# revision 1
# baseline (speedup 1.0000x reference)
"""CenterLoss kernel for Trainium2 (8 NeuronCores, SPMD data-parallel over B).

Algorithm
---------
reference computes:
    counts[c] = #{i: y_i = c};  sums[c] = sum_{i: y_i = c} f_i
    means = sums / max(counts, 1);  present = counts > 0
    n_c = present ? 0.5*centers_c + 0.5*means_c : centers_c
    loss = 0.5 * mean_i ||f_i - n_{y_i}||^2

Expanding the loss (every class that appears in the batch is present):
    B * 2 * loss = S1 - 0.5*A - 0.75*X + 0.25*W
where
    S1 = sum_i ||f_i||^2
    A  = sum_c sums_c . centers_c
    X  = sum_{c present} ||sums_c||^2 / counts_c
    W  = sum_c counts_c * ||centers_c||^2

So the only heavy device work is the segment sums/counts over feats
(B=131072, D=256, C=1000) and S1.  Each core takes B/8 rows and computes:
  - partial segment sums+counts via one-hot matmuls on the PE
    (one-hot built on DVE from an iota table, feats converted fp32->fp16 on
    ACT; counts ride along as a 257th all-ones column of the rhs)
  - partial S1 via ACT Square activation with free-dim accumulation
The host sums the 8 partial [1024,257] tensors + 8 partial S1 vectors and
evaluates the tiny [C,D] closed form above (the gather/unshard step).
"""

import sys

sys.path.insert(0, "/opt/trn_rl_repo")

import numpy as np

# problem shape (hardcoded per the harness contract)
B, D, C = 131072, 256, 1000
N_CORES = 8
BS = B // N_CORES  # 16384 rows per core
P = 128
G = 4  # row-tiles per DMA group
TILES = BS // P  # 128
GROUPS = TILES // G  # 32
CPAD = 1024  # padded class count
CCHUNKS = CPAD // P  # 8
NFREE = D + 1  # 256 feat cols + 1 ones col for counts
FSTRIDE = 264  # fp16 sub-tile stride (4B aligned, 16B padded)
TAILG = 4  # trailing groups processed chunk-outer (store/compute overlap)

_CACHE: dict = {}


def _build_program():
    import concourse.bacc as bacc
    import concourse.bass as bass
    from concourse import mybir
    from concourse.tile import TileContext

    nc = bacc.Bacc("TRN2", target_bir_lowering=False)

    feats = nc.dram_tensor("feats", [BS, D], mybir.dt.float32, kind="ExternalInput")
    labels_in = nc.dram_tensor(
        "labels", [P, TILES], mybir.dt.float16, kind="ExternalInput"
    )
    # [128 x (8*257 sums+counts | 1 s1)]; stored per chunk so early stores
    # overlap the tail matmuls
    out_sums = nc.dram_tensor(
        "out_sums", [P, CCHUNKS * NFREE + 1], mybir.dt.float32, kind="ExternalOutput"
    )

    feats_ap = feats[:]

    with TileContext(nc) as tc:
        with (
            tc.tile_pool(name="const", bufs=1) as const,
            tc.tile_pool(name="fin", bufs=4) as fin,
            tc.tile_pool(name="sq", bufs=2) as sqp,
            tc.tile_pool(name="f16p", bufs=TAILG + 2) as f16p,
            tc.tile_pool(name="ohp", bufs=4 * TAILG + 6) as ohp,
            tc.tile_pool(name="accp", bufs=1) as accp,
            tc.tile_pool(name="psp", bufs=1, space="PSUM") as psp,
        ):
            # labels DMA (fp16, converted to fp32 on DVE: tensor_scalar
            # is_equal needs an fp32 scalar operand); iota built on the
            # otherwise-idle GPSIMD engine, converted int32 -> fp16 on DVE
            labels16_t = const.tile([P, TILES], mybir.dt.float16, tag="labels16_t")
            nc.sync.dma_start(out=labels16_t[:], in_=labels_in[:])
            labels_t = const.tile([P, TILES], mybir.dt.float32, tag="labels_t")
            nc.vector.tensor_copy(out=labels_t[:], in_=labels16_t[:])
            iota_i = const.tile([P, CPAD], mybir.dt.int32, tag="iota_i")
            nc.gpsimd.iota(iota_i[:], pattern=[[1, CPAD]], channel_multiplier=0)
            iota_f = const.tile([P, CPAD], mybir.dt.float16, tag="iota_f")
            nc.vector.tensor_copy(out=iota_f[:], in_=iota_i[:])
            iota_t = iota_f[:]

            tail_ohs, tail_f16gs = [], []
            # persistent accumulators
            # one column per (group, extra-half): 32 + 3 split extras
            s1cols = accp.tile([P, GROUPS + 3], mybir.dt.float32, tag="s1cols")
            s1_extra_col = [GROUPS]  # next free extra column
            psums = [
                psp.tile(
                    [P, NFREE], mybir.dt.float32, tag=f"psum{k}", name=f"psum{k}"
                )
                for k in range(CCHUNKS)
            ]
            # HAM warm-up: the PE runs at the cold 1.2 GHz clock until ~3.4us
            # of sustained activity. The head leaves PE idle until ~4.6us, so
            # the first ~19 real matmuls would run at half clock. Issue dummy
            # matmuls (zeroed operands, results discarded by the real
            # start=True PSUM clear) from ~0.5us so the real stream is warm.
            warm = const.tile([P, NFREE], mybir.dt.float16, tag="warm")
            nc.vector.memset(warm[:1, :1], 0.0)  # touch so Tile allocates it
            for w in range(12):
                nc.tensor.matmul(
                    out=psums[0][:],
                    lhsT=warm[:, 0:P],
                    rhs=warm[:],
                    start=True,
                    stop=True,
                )

            for t in range(GROUPS):
                # load a [P, G, D] group of feats rows (rows t*512 .. t*512+511).
                # Groups 0/1 are split into smaller loads/conversions so the
                # first matmul starts as soon as the first 128 rows land.
                f16g = f16p.tile([P, G, FSTRIDE], mybir.dt.float16, tag="f16g")
                if t == 0:
                    halves = ((0, 1), (1, 1), (2, 2))
                elif t == 1:
                    halves = ((0, 2), (2, 2))
                else:
                    halves = ((0, G),)
                for h, (off, gh) in enumerate(halves):
                    fg = fin.tile(
                        [P, gh, D], mybir.dt.float32, tag="fg", name="fg"
                    )
                    # very first load rides the ACT HWDGE ring so its
                    # descriptor-gen overlaps the labels DMA's on the SP ring
                    dma_eng = nc.scalar if t == 0 else nc.sync
                    dma_eng.dma_start(
                        out=fg[:],
                        in_=bass.AP(
                            tensor=feats_ap.tensor,
                            offset=(t * G + off) * P * D,
                            ap=[[D, P], [P * D, gh], [1, D]],
                        ),
                    )
                    # fp32 -> fp16 conversion (ACT)
                    nc.scalar.copy(
                        out=f16g[:, off : off + gh, 0:D], in_=fg[:]
                    )
                    # S1 partial: sum over free dim of feats^2 (ACT square+accum)
                    sqt = sqp.tile([P, gh, D], mybir.dt.float32, tag="sqt", name="sqt")
                    if h == 0:
                        col = t
                    else:
                        col = s1_extra_col[0]
                        s1_extra_col[0] += 1
                    nc.scalar.activation(
                        out=sqt[:],
                        in_=fg[:],
                        func=mybir.ActivationFunctionType.Square,
                        accum_out=s1cols[:, col : col + 1],
                    )
                # ones column for counts (DVE)
                nc.vector.memset(f16g[:, :, D : D + 1], 1.0)

                ohs = []
                for s in range(G):
                    j = t * G + s
                    oh = ohp.tile([P, CPAD], mybir.dt.float16, tag="oh")
                    nc.vector.tensor_scalar(
                        oh[:],
                        iota_t,
                        labels_t[:, j : j + 1],
                        None,
                        mybir.AluOpType.is_equal,
                    )
                    ohs.append(oh)
                if t < GROUPS - TAILG:
                    for s in range(G):
                        rhs = f16g[:, s, 0:NFREE]
                        for k in range(CCHUNKS):
                            nc.tensor.matmul(
                                out=psums[k][:],
                                lhsT=ohs[s][:, k * P : (k + 1) * P],
                                rhs=rhs,
                                start=(t == 0 and s == 0),
                                stop=False,
                            )
                else:
                    tail_ohs.append(ohs)
                    tail_f16gs.append(f16g)
            # last TAILG groups: chunk-outer order so chunk k's accumulation
            # closes early and its evacuation/store overlaps the remaining
            # chunks' matmuls
            for k in range(CCHUNKS):
                for g, (ohs_g, f16g_g) in enumerate(zip(tail_ohs, tail_f16gs)):
                    for s in range(G):
                        nc.tensor.matmul(
                            out=psums[k][:],
                            lhsT=ohs_g[s][:, k * P : (k + 1) * P],
                            rhs=f16g_g[:, s, 0:NFREE],
                            start=False,
                            stop=(g == TAILG - 1 and s == G - 1),
                        )

            # write back partials (PSUM -> SBUF -> DRAM; DMA can't read PSUM)
            ev = accp.tile([P, CCHUNKS * NFREE + 1], mybir.dt.float32, tag="ev")
            nc.vector.tensor_reduce(
                out=ev[:, CCHUNKS * NFREE : CCHUNKS * NFREE + 1],
                in_=s1cols[:],
                axis=mybir.AxisListType.X,
                op=mybir.AluOpType.add,
            )
            for k in range(CCHUNKS):
                dst = ev[:, k * NFREE : (k + 1) * NFREE]
                if k % 2 == 0:
                    nc.vector.tensor_copy(out=dst, in_=psums[k][:])
                else:
                    nc.scalar.copy(out=dst, in_=psums[k][:])
            # per-chunk stores: chunks close ~1.7 us apart (chunk-outer tail),
            # so early stores hide under compute and the last piece is small
            for k in range(CCHUNKS):
                lo = k * NFREE
                hi = (k + 1) * NFREE + (1 if k == CCHUNKS - 1 else 0)
                nc.sync.dma_start(out=out_sums[:, lo:hi], in_=ev[:, lo:hi])

    nc.compile()
    return nc


def _get_program():
    if "nc" not in _CACHE:
        _CACHE["nc"] = _build_program()
    return _CACHE["nc"]


def _run_device(feats_np: np.ndarray, labels_np: np.ndarray, trace: bool = False):
    """Shard over cores, run the SPMD bass kernel, return per-core results."""
    from concourse.bass_utils import run_bass_kernel_spmd

    nc = _get_program()
    in_maps = []
    for c in range(N_CORES):
        fshard = np.ascontiguousarray(feats_np[c * BS : (c + 1) * BS])
        lshard = labels_np[c * BS : (c + 1) * BS]
        # [P, TILES]; fp16 is exact for labels < 2048
        ltile = np.ascontiguousarray(lshard.reshape(TILES, P).T.astype(np.float16))
        in_maps.append({"feats": fshard, "labels": ltile})
    kw = {}
    if trace:
        kw = {"trace": True}
    try:
        return run_bass_kernel_spmd(nc, in_maps, core_ids=list(range(N_CORES)), **kw)
    except Exception:
        # transient axon/terminal faults have been observed; retry once
        import time

        time.sleep(2.0)
        return run_bass_kernel_spmd(nc, in_maps, core_ids=list(range(N_CORES)), **kw)


def kernel(feats, centers, labels, _trace: bool = False, _return_res: bool = False):
    feats = np.asarray(feats, dtype=np.float32)
    centers = np.asarray(centers, dtype=np.float32)
    labels_i = np.asarray(labels).astype(np.int64)

    res = _run_device(feats, labels_i, trace=_trace)

    # host combine (the gather/unshard step): tiny [C, D] math
    sums_all = np.zeros((CPAD, NFREE), dtype=np.float64)
    S1 = 0.0
    for c in range(N_CORES):
        raw = res.results[c]["out_sums"]
        part = (
            raw[:, : CCHUNKS * NFREE]
            .reshape(P, CCHUNKS, NFREE)
            .transpose(1, 0, 2)
            .reshape(CPAD, NFREE)
        )
        sums_all += part.astype(np.float64)
        S1 += float(raw[:, CCHUNKS * NFREE].sum())
    sums = sums_all[:C, :D]
    counts = sums_all[:C, D]

    c64 = centers.astype(np.float64)
    A = float((sums * c64).sum())
    present = counts > 0
    X = float((np.square(sums).sum(axis=1)[present] / counts[present]).sum())
    W = float((counts * np.square(c64).sum(axis=1)).sum())
    loss = 0.5 / B * (S1 - 0.5 * A - 0.75 * X + 0.25 * W)
    out = np.float32(loss)
    if _return_res:
        return out, res
    return out



# revision 16
# speedup vs baseline: 2.1118x; 2.1118x over previous
"""CenterLoss kernel for Trainium2 (8 NeuronCores, SPMD data-parallel over B).

Algorithm
---------
reference computes:
    counts[c] = #{i: y_i = c};  sums[c] = sum_{i: y_i = c} f_i
    means = sums / max(counts, 1);  present = counts > 0
    n_c = present ? 0.5*centers_c + 0.5*means_c : centers_c
    loss = 0.5 * mean_i ||f_i - n_{y_i}||^2

Expanding the loss (every class that appears in the batch is present):
    B * 2 * loss = S1 - 0.5*A - 0.75*X + 0.25*W
where
    S1 = sum_i ||f_i||^2
    A  = sum_c sums_c . centers_c
    X  = sum_{c present} ||sums_c||^2 / counts_c
    W  = sum_c counts_c * ||centers_c||^2

Device work per core (16384 rows; the DMA stream of the 16.8MB fp32
feats shard at ~360GB/s is the ~47us roofline, and every other engine
is kept below it):
  - segment sums via one-hot matmuls on the PE in fp8e4 DoubleRow perf
    mode (0.5 cycles/out-row, double contraction = 4x the fp16 rate).
    The one-hot is built on the DVE as fp16 is_equal output (the 4x_2p
    DVE mode needs 2-byte dtypes) and *bitcast* to fp8: fp16 1.0 =
    0x3C00 whose little-endian bytes are (0x00, 0x3C), i.e. an exact
    fp8e4m3 1.5 at every odd byte.  The matmul weights AP reads the odd
    bytes with stride 2, so the DVE never pays the 1-byte-dtype penalty
    and the host divides the sums by 1.5.
  - feats are converted fp32 -> fp8e4 on ACT (the only ACT bulk work).
  - S1 comes from a PE Gram accumulation (lhsT = rhs = fp8 feats, take
    the diagonal on host); the fp8 quantization biases S1 by ~0.1%,
    well inside the 2e-2 tolerance.
  - counts are exact via host-side bincount of labels (host already
    preprocesses labels into per-core tiles).
Tail latency (after the last feats byte lands) is the only part above
the roofline, so it is aggressively trimmed: the last two groups'
conversions are split across ACT and DVE, only the final DoubleRow
pair is deferred (chunk-outer), psum banks are evacuated on three
engines (ACT/DVE/GPSIMD) into one byte-packed output tile (gram as
fp16, sums chunks as fp8 - quantizing the *sums* costs ~0.1% bias),
and the result leaves in two stores (HWDGE descriptor generation is
~625ns per DMA, so fewer, bigger stores win).  The host sums the 8
partial [1024,256] tensors + Gram diagonals and evaluates the tiny
[C,D] closed form above (the gather/unshard step).
"""

import sys

sys.path.insert(0, "/opt/trn_rl_repo")

import numpy as np

# problem shape (hardcoded per the harness contract)
B, D, C = 131072, 256, 1000
N_CORES = 8
BS = B // N_CORES  # 16384 rows per core
P = 128
G = 4  # row-tiles per group (512 rows, two DoubleRow pairs)
TILES = BS // P  # 128
GROUPS = TILES // G  # 32
CPAD = 1024  # padded class count
CCHUNKS = CPAD // P  # 8
OH_SCALE = 1.5  # fp8 value at the odd byte of an fp16 1.0
GRAM_B = 4 * D  # gram as fp16: 512 cols * 2 bytes
NOUT_B = GRAM_B + CPAD * D // P  # + 8 fp8 sums chunks of 256 bytes
STORE_SPLIT = GRAM_B + 4 * D  # store1: gram + chunks 0-3; store2: 4-7

_CACHE: dict = {}


def _build_program():
    import concourse.bacc as bacc
    import concourse.bass as bass
    from concourse import mybir
    from concourse.tile import TileContext

    DR = mybir.MatmulPerfMode.DoubleRow
    f8 = mybir.dt.float8e4

    nc = bacc.Bacc("TRN2", target_bir_lowering=False)

    feats = nc.dram_tensor("feats", [BS, D], mybir.dt.float32, kind="ExternalInput")
    labels_in = nc.dram_tensor(
        "labels", [P, TILES], mybir.dt.float16, kind="ExternalInput"
    )
    # fp16-typed output (the jax/axon bridge rejects fp8 I/O buffers); the
    # byte layout is [gram fp16 | sums chunks fp8], decoded on the host
    out_sums = nc.dram_tensor(
        "out_sums", [P, NOUT_B // 2], mybir.dt.float16, kind="ExternalOutput"
    )

    feats_ap = feats[:]

    with TileContext(nc) as tc:
        with (
            tc.tile_pool(name="const", bufs=1) as const,
            tc.tile_pool(name="fin", bufs=6) as fin,
            tc.tile_pool(name="f8p", bufs=4) as f8p,
            tc.tile_pool(name="ohp", bufs=4) as ohp,
            tc.tile_pool(name="evp", bufs=1) as evp,
            tc.tile_pool(name="psp", bufs=1, space="PSUM") as psp,
        ):
            # the very first feats load is emitted before anything else so
            # its HWDGE descriptor generation wins the shared generator;
            # labels ride the ACT ring right behind it.  fp16 labels are
            # converted to fp32 on DVE (tensor_scalar is_equal needs an fp32
            # scalar operand); iota is built int16 on the otherwise-idle
            # GPSIMD engine and converted to fp16 on DVE.
            fg0 = fin.tile([P, 2, D], mybir.dt.float32, tag="fg", name="fg")
            nc.sync.dma_start(
                out=fg0[:],
                in_=bass.AP(
                    tensor=feats_ap.tensor,
                    offset=0,
                    ap=[[D, P], [P * D, 2], [1, D]],
                ),
            )
            labels16_t = const.tile([P, TILES], mybir.dt.float16, tag="labels16_t")
            nc.scalar.dma_start(out=labels16_t[:], in_=labels_in[:])
            labels_t = const.tile([P, TILES], mybir.dt.float32, tag="labels_t")
            nc.vector.tensor_copy(out=labels_t[:], in_=labels16_t[:])
            iota_i = const.tile([P, CPAD], mybir.dt.int16, tag="iota_i")
            nc.gpsimd.iota(iota_i[:], pattern=[[1, CPAD]], channel_multiplier=0)
            iota_f = const.tile([P, CPAD], mybir.dt.float16, tag="iota_f")
            nc.vector.tensor_copy(out=iota_f[:], in_=iota_i[:])
            iota_t = iota_f[:]

            # persistent accumulators: 4 banks of two 256-col sums chunks
            # (chunk k lives in bank k//2, half k%2) + one gram bank
            psums = [
                psp.tile([P, 2 * D], mybir.dt.float32, tag=f"psum{j}", name=f"psum{j}")
                for j in range(CCHUNKS // 2)
            ]
            gram = psp.tile([P, 2 * D], mybir.dt.float32, tag="gram", name="gram")

            def sum_out(k):
                return psums[k // 2][:, (k % 2) * D : (k % 2) * D + D]

            # HAM warm-up: the PE runs at the cold clock until ~3us of
            # sustained activity; the head leaves PE idle until ~4us.
            # Dummy matmuls (zeroed operands, results discarded by the real
            # start=True PSUM clear) keep the real stream warm.
            warm = const.tile([P, 2 * D], mybir.dt.float16, tag="warm")
            nc.vector.memset(warm[:1, :1], 0.0)  # touch so Tile allocates it
            for _ in range(4):
                nc.tensor.matmul(
                    out=psums[0][:],
                    lhsT=warm[:, 0:P],
                    rhs=warm[:],
                    start=True,
                    stop=True,
                )

            started = [False]

            def pair_matmuls(oh8, f8g, q, do_gram):
                rhs = f8g[:, 2 * q : 2 * q + 2, :]
                for k in range(CCHUNKS):
                    nc.tensor.matmul(
                        out=sum_out(k),
                        lhsT=oh8[
                            :, 2 * q : 2 * q + 2, 2 * P * k + 1 : 2 * P * (k + 1) : 2
                        ],
                        rhs=rhs,
                        start=not started[0],
                        stop=False,
                        perf_mode=DR,
                    )
                if do_gram:
                    for k in range(2):
                        nc.tensor.matmul(
                            out=gram[:, k * D : (k + 1) * D],
                            lhsT=f8g[:, 2 * q : 2 * q + 2, k * P : (k + 1) * P],
                            rhs=rhs,
                            start=not started[0],
                            stop=False,
                            perf_mode=DR,
                        )
                started[0] = True

            tail = []
            for t in range(GROUPS):
                # load a [P, G, D] group of feats rows (rows t*512..t*512+511);
                # the first group's load is split (emitted above) so the
                # pipeline head fills early, the last group is quartered so
                # its final conversion (on the end-to-end critical path) is
                # tiny.  The last two groups' conversions are split across
                # ACT and DVE so the tail's ACT queue stays short.
                f8g = f8p.tile([P, G, D], f8, tag="f8g")
                if t == 0:
                    pieces = [(0, 2, nc.scalar, fg0), (2, 2, nc.scalar, None)]
                elif t == GROUPS - 3:
                    pieces = [(0, 2, nc.scalar, None), (2, 2, nc.gpsimd, None)]
                elif t == GROUPS - 2:
                    pieces = [(0, 2, nc.scalar, None), (2, 2, nc.vector, None)]
                elif t == GROUPS - 1:
                    pieces = [
                        (0, 2, nc.scalar, None),
                        (2, 1, nc.vector, None),
                        (3, 1, nc.scalar, None),
                    ]
                else:
                    pieces = [(0, G, nc.scalar, None)]
                for off, gh, cvt_eng, preloaded in pieces:
                    if preloaded is not None:
                        fg = preloaded
                    else:
                        fg = fin.tile(
                            [P, gh, D], mybir.dt.float32, tag="fg", name="fg"
                        )
                        nc.sync.dma_start(
                            out=fg[:],
                            in_=bass.AP(
                                tensor=feats_ap.tensor,
                                offset=(t * G + off) * P * D,
                                ap=[[D, P], [P * D, gh], [1, D]],
                            ),
                        )
                    # fp32 -> fp8e4 conversion
                    if cvt_eng is nc.scalar:
                        nc.scalar.copy(out=f8g[:, off : off + gh, :], in_=fg[:])
                    else:
                        nc.vector.tensor_copy(
                            out=f8g[:, off : off + gh, :], in_=fg[:]
                        )
                # one-hots for the 4 row-tiles (DVE, fp16 out = 4x mode)
                oh = ohp.tile([P, G, CPAD], mybir.dt.float16, tag="oh")
                for s in range(G):
                    j = t * G + s
                    nc.vector.tensor_scalar(
                        oh[:, s, :],
                        iota_t,
                        labels_t[:, j : j + 1],
                        None,
                        mybir.AluOpType.is_equal,
                    )
                oh8 = oh[:].bitcast(f8)

                if t < GROUPS - 1:
                    for q in range(G // 2):
                        pair_matmuls(oh8, f8g, q, do_gram=True)
                else:
                    # the last pair is deferred (chunk-outer tail); pair 0
                    # still flows through the main pipeline
                    pair_matmuls(oh8, f8g, 0, do_gram=True)
                    tail.append((oh8, f8g))

            # tail (last pair only): close gram first, then one chunk-closing
            # matmul each; per-bank [P,512] evacuations on three engines
            # (ACT/DVE/GPSIMD) into a byte-packed ev tile (gram fp16, sums
            # chunks fp8), leaving in two batched stores.
            ev = evp.tile([P, NOUT_B], f8, tag="ev")
            ev16 = ev[:].bitcast(mybir.dt.float16)
            (oh8, f8g) = tail[0]
            rhs = f8g[:, 2:4, :]
            for k in range(2):
                nc.tensor.matmul(
                    out=gram[:, k * D : (k + 1) * D],
                    lhsT=f8g[:, 2:4, k * P : (k + 1) * P],
                    rhs=rhs,
                    start=False,
                    stop=(k == 1),
                    perf_mode=DR,
                )
            nc.scalar.copy(out=ev16[:, 0 : 2 * D], in_=gram[:])
            # GPSIMD cannot read PSUM (BIR verifier), so only ACT/DVE evacuate
            evac_eng = {0: "vector", 1: "scalar", 2: "vector", 3: "scalar"}
            for k in range(CCHUNKS):
                nc.tensor.matmul(
                    out=sum_out(k),
                    lhsT=oh8[:, 2:4, 2 * P * k + 1 : 2 * P * (k + 1) : 2],
                    rhs=rhs,
                    start=False,
                    stop=True,
                    perf_mode=DR,
                )
                if k % 2 == 1:
                    j = k // 2  # bank j = chunks 2j, 2j+1
                    dst = ev[:, GRAM_B + 2 * j * D : GRAM_B + 2 * (j + 1) * D]
                    eng = getattr(nc, evac_eng[j])
                    if evac_eng[j] == "scalar":
                        eng.copy(out=dst, in_=psums[j][:])
                    else:
                        eng.tensor_copy(out=dst, in_=psums[j][:])
                    if j == 1:
                        nc.sync.dma_start(
                            out=out_sums[:, : STORE_SPLIT // 2],
                            in_=ev16[:, : STORE_SPLIT // 2],
                        )
            nc.sync.dma_start(
                out=out_sums[:, STORE_SPLIT // 2 :], in_=ev16[:, STORE_SPLIT // 2 :]
            )

    nc.compile()
    return nc


def _get_program():
    if "nc" not in _CACHE:
        _CACHE["nc"] = _build_program()
    return _CACHE["nc"]


def _run_device(feats_np: np.ndarray, labels_np: np.ndarray, trace: bool = False):
    """Shard over cores, run the SPMD bass kernel, return per-core results."""
    from concourse.bass_utils import run_bass_kernel_spmd

    nc = _get_program()
    in_maps = []
    for c in range(N_CORES):
        fshard = np.ascontiguousarray(feats_np[c * BS : (c + 1) * BS])
        lshard = labels_np[c * BS : (c + 1) * BS]
        # [P, TILES]; fp16 is exact for labels < 2048
        ltile = np.ascontiguousarray(lshard.reshape(TILES, P).T.astype(np.float16))
        in_maps.append({"feats": fshard, "labels": ltile})
    kw = {}
    if trace:
        kw = {"trace": True}
    try:
        return run_bass_kernel_spmd(nc, in_maps, core_ids=list(range(N_CORES)), **kw)
    except Exception:
        # transient axon/terminal faults have been observed; retry once
        import time

        time.sleep(2.0)
        return run_bass_kernel_spmd(nc, in_maps, core_ids=list(range(N_CORES)), **kw)


def kernel(feats, centers, labels, _trace: bool = False, _return_res: bool = False):
    feats = np.asarray(feats, dtype=np.float32)
    centers = np.asarray(centers, dtype=np.float32)
    labels_i = np.asarray(labels).astype(np.int64)

    res = _run_device(feats, labels_i, trace=_trace)

    # host combine (the gather/unshard step): tiny [C, D] math
    from concourse import mybir

    f8_np = mybir.dt.np(mybir.dt.float8e4)
    sums_all = np.zeros((CPAD, D), dtype=np.float64)
    S1 = 0.0
    for c in range(N_CORES):
        raw16 = np.asarray(res.results[c]["out_sums"])  # [P, NOUT_B//2] fp16
        by = np.frombuffer(raw16.tobytes(), dtype=np.uint8).reshape(P, NOUT_B)
        g = (
            np.frombuffer(by[:, :GRAM_B].tobytes(), dtype=np.float16)
            .reshape(P, 2 * D)
            .astype(np.float64)
        )  # [P, 512] gram
        S1 += float(np.trace(g[:, 0:P])) + float(np.trace(g[:, D + P : 2 * D]))
        part = (
            np.frombuffer(by[:, GRAM_B:].tobytes(), dtype=f8_np)
            .reshape(P, CCHUNKS, D)
            .astype(np.float64)
            .transpose(1, 0, 2)
            .reshape(CPAD, D)
        )
        sums_all += part
    sums = sums_all[:C, :] / OH_SCALE
    counts = np.bincount(labels_i, minlength=C).astype(np.float64)

    c64 = centers.astype(np.float64)
    A = float((sums * c64).sum())
    present = counts > 0
    X = float((np.square(sums).sum(axis=1)[present] / counts[present]).sum())
    W = float((counts * np.square(c64).sum(axis=1)).sum())
    loss = 0.5 / B * (S1 - 0.5 * A - 0.75 * X + 0.25 * W)
    out = np.float32(loss)
    if _return_res:
        return out, res
    return out
